# revision 14
# baseline (speedup 1.0000x reference)
"""Multi-head self-attention (RoPE, causal) on 8 trn2 NeuronCores.

Sharding: core c -> batch b = c // 4, head group g = c % 4 (4 heads each).

Per-core design (v2, W-stationary projection):
  - QKV projection runs W-stationary: lhsT = W^T chunk [128 d, 128 cols],
    rhs = x^T [128 d, 512 tokens], so Q^T/K^T/V^T come out already
    transposed ([feature, token]).  W columns are ordered per head-pair
    [q01 k01 v01 q23 k23 v23] and Q/K features are host-permuted to
    rotate-half order (even dims first), so RoPE becomes full-tile
    multiplies with per-token cos/sin tables (host-computed from
    token_positions - no gather) plus +-32-partition swaps.
  - scores^T = K^T-chunk x Q (k on partitions) into [128,1024] PSUM
    tiles, exp on ACT -> per-j et tiles (SBUF, column-offset 128j),
    diagonal masked by a lower-tri multiply.
  - V'-stationary matmuls (V plus a ones-column so softmax sums ride as
    row 64) accumulate DIRECTLY IN PSUM per 512-token q-chunk; only two
    q-chunk banks are live at a time (qc0/qc1 open first; when one
    closes it is normalized and the next opens, replaying buffered et
    tiles).  Normalize = DVE reciprocal of the sums row + gpsimd
    partition-broadcast + one multiply -> attT.
  - Projection of heads 2,3 + V transposes + out-projection pass 1 are
    emitted as PE filler between score/V-matmul steps, since the
    attention phase is ACT(exp)-paced.
  - AllGather (4-core batch group, split per head-pair) of attT; a
    per-core index gather (wsel) extracts this core's 512-token
    q-slice; two-pass output projection with the first AG consumed
    early.
Host only reshapes/permutes/casts inputs and concatenates outputs.
"""

import sys

for _p in ("/opt/trn_rl_repo",):
    if _p not in sys.path:
        sys.path.append(_p)

import numpy as np
import ml_dtypes

import concourse.bass as bass
import concourse.mybir as mybir
import concourse.tile as tile
from concourse import bacc
from concourse.bass import ds, ts
from concourse.bass_utils import run_bass_kernel_spmd
from concourse.masks import make_identity

BF16 = mybir.dt.bfloat16
F32 = mybir.dt.float32
I32 = mybir.dt.int32

B, S, D = 2, 2048, 1024
H, DK = 16, 64
THETA = 10000.0
N_CORES = 8
GROUPS = 4          # head groups (cores) per batch
HPC = H // GROUPS   # heads per core = 4
NSC = S // 128      # 16 token chunks
NQC = S // 512      # 4 q column-chunks
QSLICE = S // GROUPS
MUL = mybir.AluOpType.mult
ADD = mybir.AluOpType.add
EXP = mybir.ActivationFunctionType.Exp


def _build():
    nc = bacc.Bacc("TRN2", num_devices=N_CORES)

    xT = nc.dram_tensor("xT", [D, S], BF16, kind="ExternalInput")
    wqkvT = nc.dram_tensor("wqkvT", [D, 6 * 128], BF16, kind="ExternalInput")
    woT = nc.dram_tensor("woT", [D, D], BF16, kind="ExternalInput")
    costab = nc.dram_tensor("costab", [128, S], BF16, kind="ExternalInput")
    sintab = nc.dram_tensor("sintab", [128, S], BF16, kind="ExternalInput")
    tri = nc.dram_tensor("tri", [128, 128], BF16, kind="ExternalInput")
    wsel = nc.dram_tensor("wsel", [QSLICE, 1], I32, kind="ExternalInput")
    finT = nc.dram_tensor("finT", [D, QSLICE], F32, kind="ExternalOutput")

    with tile.TileContext(nc) as tc:
        with (
            tc.tile_pool(name="const", bufs=1) as constp,
            tc.tile_pool(name="wts", bufs=1) as wtsp,
            tc.tile_pool(name="seq", bufs=1) as seqp,
            tc.tile_pool(name="ropet", bufs=2) as ropet,
            tc.tile_pool(name="ett", bufs=1) as ett,
            tc.tile_pool(name="attp", bufs=1) as attp,
            tc.tile_pool(name="nrm", bufs=2) as nrmp,
            tc.tile_pool(name="xtp", bufs=3) as xtp,
            tc.tile_pool(name="spp", bufs=3, space="PSUM") as spp,
            tc.tile_pool(name="partp", bufs=2, space="PSUM") as partp,
            tc.tile_pool(name="dram", bufs=1, space="DRAM") as dramp,
        ):
            # ---------------- constants + resident weights ----------------
            ident = constp.tile([128, 128], BF16)
            make_identity(nc, ident[:])
            tri_t = constp.tile([128, 128], BF16)
            nc.sync.dma_start(out=tri_t[:], in_=tri[:])

            wt = wtsp.tile([128, 8, 6, 128], BF16)   # W^T [dchunk, wchunk]
            for wc in (0, 1):
                nc.sync.dma_start(
                    out=wt[:, :, wc, :],
                    in_=wqkvT[:, ts(wc, 128)].rearrange(
                        "(k p) c -> p k c", p=128),
                )
            xa = wtsp.tile([128, 8, S], BF16)     # resident x^T
            for k in range(8):
                nc.sync.dma_start(
                    out=xa[:, k, 0:1024], in_=xT[ts(k, 128), 0:1024])
            cost = wtsp.tile([128, S], BF16)
            nc.sync.dma_start(out=cost[:], in_=costab[:])
            sint = wtsp.tile([128, S], BF16)
            nc.sync.dma_start(out=sint[:], in_=sintab[:])
            for k in range(8):
                nc.sync.dma_start(
                    out=xa[:, k, 1024:2048], in_=xT[ts(k, 128), 1024:2048])
            for wc in (2, 3, 4, 5):
                nc.sync.dma_start(
                    out=wt[:, :, wc, :],
                    in_=wqkvT[:, ts(wc, 128)].rearrange(
                        "(k p) c -> p k c", p=128),
                )
            wo = wtsp.tile([128, 8, D], BF16)
            for k in range(8):
                nc.sync.dma_start(out=wo[:, k, :], in_=woT[ts(k, 128), :])

            # persistent per-core tensors
            qt = seqp.tile([128, 2, S], BF16)   # Q^T  [pair(2x64 feat), q]
            kt = seqp.tile([128, 2, S], BF16)   # K^T
            vv = seqp.tile([128, NSC, HPC, DK + 1], BF16)  # V + ones col
            nc.vector.memset(vv[:, :, :, DK:DK + 1], 1.0)
            attT = attp.tile([128, 2, S], BF16)
            rwo = attp.tile([128, 8, QSLICE], BF16)
            fe = attp.tile([128, 8, QSLICE], F32)
            wsel_sb = attp.tile([128, 4], I32)

            # per-j exp tiles: column q stored at (q - 128j)
            et = [ett.tile([128, S - 128 * j], BF16, name=f"et{j}")
                  for j in range(NSC)]

            # ------------- building blocks -------------
            def emit_proj(wc, tb):
                """Project wchunk wc for token block tb -> PSUM tile."""
                pj = spp.tile([128, 512], F32, space="PSUM", tag="big",
                              name="pj")
                for k in range(8):
                    nc.tensor.matmul(
                        pj[:], lhsT=wt[:, k, wc, :],
                        rhs=xa[:, k, ts(tb, 512)],
                        start=(k == 0), stop=(k == 7),
                    )
                return pj

            shuf16 = [i ^ 16 for i in range(32)]

            def emit_rope(wc, tb, pj):
                """RoPE pj -> qt/kt pair (wc in {0,1,3,4}).

                Features are quadrant-packed [16 even | 16 odd] so the
                rotate partner sits 16 partitions away within each
                32-partition quadrant; stream_shuffle does the swap."""
                dst = qt if wc in (0, 3) else kt
                pr = 0 if wc in (0, 1) else 1
                xsw = ropet.tile([128, 512], F32, tag="xsw")
                nc.vector.stream_shuffle(xsw[:], pj[:], shuf16)
                a = ropet.tile([128, 512], BF16, tag="ra")
                b = ropet.tile([128, 512], BF16, tag="rb")
                nc.vector.tensor_tensor(a[:], cost[:, ts(tb, 512)], pj[:],
                                        MUL)
                nc.vector.tensor_tensor(b[:], sint[:, ts(tb, 512)], xsw[:],
                                        MUL)
                nc.vector.tensor_tensor(
                    dst[:, pr, ts(tb, 512)], a[:], b[:], ADD)

            def emit_vchunk(wc, tb):
                """Project V wchunk + transpose into vv chunks 4tb..4tb+3."""
                h0 = 0 if wc == 2 else 2
                pj = emit_proj(wc, tb)
                vT = ropet.tile([128, 512], BF16, tag="vT")
                nc.vector.tensor_copy(vT[:], pj[:])
                for t in range(4):
                    tp = spp.tile([128, 128], BF16, space="PSUM", tag="big",
                                  name="tp")
                    nc.tensor.transpose(tp[:], vT[:, ts(t, 128)], ident[:])
                    nc.vector.tensor_copy(
                        vv[:, tb * 4 + t, h0:h0 + 2, 0:DK],
                        tp[:].rearrange("p (h e) -> p h e", h=2),
                    )

            # ---- filler queue (PE work consumed inside attention) ----
            fillers = []

            def drain(n=1):
                for _ in range(min(n, len(fillers))):
                    fillers.pop(0)()

            # ------------- attention -------------
            agin = [dramp.tile([4 * 128, QSLICE], BF16, name=f"agin{p}")
                    for p in range(2)]
            agout = [dramp.tile([4 * 512, QSLICE], BF16, name=f"agout{p}")
                     for p in range(2)]

            def emit_scores(h, j):
                pr, hf = h // 2, (h % 2) * DK
                for qh in range(2):
                    q0 = max(128 * j, 1024 * qh)
                    q1 = 1024 * (qh + 1)
                    if q0 >= q1:
                        continue
                    sp = spp.tile([128, 1024], F32, space="PSUM", tag="big")
                    for qq in (1024 * qh, 1024 * qh + 512):
                        a, bnd = max(q0, qq), min(q1, qq + 512)
                        if a >= bnd:
                            continue
                        nc.tensor.matmul(
                            sp[:, ds(a - 1024 * qh, bnd - a)],
                            lhsT=kt[ds(hf, DK), pr, ts(j, 128)],
                            rhs=qt[ds(hf, DK), pr, ds(a, bnd - a)],
                            start=True, stop=True,
                        )
                    nc.scalar.activation(
                        et[j][:, ds(q0 - 128 * j, q1 - q0)],
                        sp[:, ds(q0 - 1024 * qh, q1 - q0)], EXP)
                # mask the diagonal block (q < k -> 0)
                nc.vector.tensor_tensor(
                    et[j][:, 0:128], et[j][:, 0:128], tri_t[:], MUL)

            def vmm(h, qc, j, start):
                a = max(512 * qc, 128 * j)
                bnd = 512 * (qc + 1)
                nc.tensor.matmul(
                    parts[qc][:, ds(a - 512 * qc, bnd - a)],
                    lhsT=vv[:, j, h, :],
                    rhs=et[j][:, ds(a - 128 * j, bnd - a)],
                    start=start, stop=(j == 4 * qc + 3),
                    skip_group_check=True,
                )

            def emit_normalize(h, qc):
                pr, hf = h // 2, (h % 2) * DK
                part = parts[qc]
                srow = nrmp.tile([1, 512], F32, tag="srow")
                nc.vector.tensor_copy(srow[:], part[ds(DK, 1), :])
                nc.vector.reciprocal_approx_fast(srow[:], srow[:])
                sumb = nrmp.tile([64, 512], F32, tag="sumb")
                nc.gpsimd.partition_broadcast(sumb[:], srow[:])
                nc.vector.tensor_tensor(
                    attT[ds(hf, DK), pr, ts(qc, 512)],
                    part[0:DK, :], sumb[:], MUL)
                parts[qc] = None
                if h % 2 == 1:   # pair piece complete -> stage for AG
                    p = h // 2
                    nc.sync.dma_start(
                        out=agin[p][:, :].rearrange(
                            "(d j) q -> d j q", j=GROUPS)[:, qc, :],
                        in_=attT[:, p, ts(qc, 512)],
                    )
                    if qc == NQC - 1:
                        nc.gpsimd.collective_compute(
                            "AllGather",
                            mybir.AluOpType.bypass,
                            ins=[agin[p][:]],
                            outs=[agout[p][:]],
                            replica_groups=[[0, 1, 2, 3], [4, 5, 6, 7]],
                        )

            def open_group(h, qc, jmax):
                """(Re)open accumulation group qc, replaying et j=0..jmax."""
                parts[qc] = partp.tile([DK + 1, 512], F32, space="PSUM",
                                       tag="part", name=f"part{qc}")
                for j in range(0, min(jmax, 4 * qc + 3) + 1):
                    vmm(h, qc, j, start=(j == 0))

            def emit_vstep(h, j):
                """V-matmuls for step j + group closures."""
                for qc in (0, 1, 2, 3):
                    if parts[qc] is not None and j <= 4 * qc + 3:
                        vmm(h, qc, j, start=(j == 0))
                for qc in (0, 1, 2, 3):
                    if parts[qc] is not None and j == 4 * qc + 3:
                        emit_normalize(h, qc)
                        if qc + 2 <= 3:
                            open_group(h, qc + 2, j)

            # ------------- out-projection pieces -------------
            for c in range(4):
                nc.sync.dma_start(
                    out=wsel_sb[:, c:c + 1], in_=wsel[ts(c, 128), :])

            def emit_gather(dp):
                nc.gpsimd.indirect_dma_start(
                    out=rwo[:, dp, :],
                    out_offset=None,
                    in_=agout[dp % 2][:],
                    in_offset=bass.IndirectOffsetOnAxis(
                        ap=wsel_sb[:, dp // 2:dp // 2 + 1], axis=0),
                )

            def emit_pass1(ec):
                fp = spp.tile([128, 512], F32, space="PSUM", tag="big")
                for i, dp in enumerate((0, 2, 4, 6)):
                    nc.tensor.matmul(
                        fp[:], lhsT=wo[:, dp, ts(ec, 128)], rhs=rwo[:, dp, :],
                        start=(i == 0), stop=(i == 3),
                    )
                nc.vector.tensor_copy(fe[:, ec, :], fp[:])

            # ================= emission schedule =================
            # Phase A: Q,K of heads 0,1 (wc 0,1) through the big PSUM pool
            for wc in (0, 1):
                for tb in range(4):
                    emit_rope(wc, tb, emit_proj(wc, tb))

            # fillers: q23, k23 rope quanta then v23 chunks
            for wc in (3, 4):
                for tb in range(4):
                    fillers.append(
                        lambda wc=wc, tb=tb: emit_rope(wc, tb,
                                                       emit_proj(wc, tb)))

            for h in range(HPC):
                parts = [None, None, None, None]
                open_group(h, 0, -1)
                open_group(h, 1, -1)
                for j in range(NSC):
                    if h == 0 and j % 4 == 0:
                        emit_vchunk(2, j // 4)
                    if h == 2 and j % 4 == 0:
                        emit_vchunk(5, j // 4)
                    emit_scores(h, j)
                    if j > 0:
                        emit_vstep(h, j - 1)
                    if h in (0, 1) and j % 2 == 1:
                        drain(1)
                    if h in (2, 3) and j % 2 == 1 and j > 3:
                        drain(1)
                emit_vstep(h, NSC - 1)
                for qc in (0, 1, 2, 3):   # close any remaining groups
                    if parts[qc] is not None:
                        emit_normalize(h, qc)
                if h == 1:
                    # AG#0 just queued; gathers for even d-chunks + pass1
                    for dp in (0, 2, 4, 6):
                        emit_gather(dp)
                    for ec in range(8):
                        fillers.append(lambda ec=ec: emit_pass1(ec))

            drain(len(fillers))

            # ------------- tail: AG#1 + pass 2 -------------
            for dp in (1, 3, 5, 7):
                emit_gather(dp)
            for ec in range(8):
                fp = spp.tile([128, 512], F32, space="PSUM", tag="big")
                for i, dp in enumerate((1, 3, 5, 7)):
                    nc.tensor.matmul(
                        fp[:], lhsT=wo[:, dp, ts(ec, 128)], rhs=rwo[:, dp, :],
                        start=(i == 0), stop=(i == 3),
                    )
                fin_sb = xtp.tile([128, QSLICE], F32, tag="fin")
                nc.vector.tensor_tensor(fin_sb[:], fe[:, ec, :], fp[:], ADD)
                nc.sync.dma_start(out=finT[ts(ec, 128), :], in_=fin_sb[:])

    nc.compile()
    return nc


def _host_prep(x, token_positions, W_qkv, W_o):
    bf16 = ml_dtypes.bfloat16
    xT = np.ascontiguousarray(np.transpose(x, (0, 2, 1))).astype(bf16)

    # quadrant-packed rotate permutation: per 32-row quadrant the even
    # dims of 16 pairs then their odd partners (stream_shuffle swaps at
    # +-16 partitions within a quadrant)
    perm64 = np.concatenate([
        np.arange(0, 32, 2), np.arange(1, 32, 2),
        np.arange(32, 64, 2), np.arange(33, 64, 2),
    ])

    wq = W_qkv[0 * D:1 * D] * np.float32(1.0 / np.sqrt(DK))
    wk = W_qkv[1 * D:2 * D]
    wv = W_qkv[2 * D:3 * D]
    wslices = []
    for g in range(GROUPS):
        base = g * HPC * DK
        cols = []
        for pr in range(2):           # head pairs
            hA, hB = base + (2 * pr) * DK, base + (2 * pr + 1) * DK
            qrows = np.concatenate(
                [wq[hA + perm64], wq[hB + perm64]], axis=0)      # [128, D]
            krows = np.concatenate(
                [wk[hA + perm64], wk[hB + perm64]], axis=0)
            vrows = np.concatenate(
                [wv[hA:hA + DK], wv[hB:hB + DK]], axis=0)
            cols += [qrows, krows, vrows]
        wsl = np.concatenate(cols, axis=0)                       # [768, D]
        wslices.append(np.ascontiguousarray(wsl.T).astype(bf16))  # [D, 768]

    woT = np.ascontiguousarray(W_o.T).astype(bf16)

    # per-token cos/sin tables in quadrant-packed transposed layout
    idx = np.arange(DK // 2, dtype=np.float64)
    freqs = 1.0 / (THETA ** (2.0 * idx / DK))                    # [32]
    pos = np.asarray(token_positions).astype(np.float64)         # [B, S]
    ang = pos[:, None, :] * freqs[None, :, None]                 # [B, 32, S]
    cos32 = np.cos(ang)
    sin32 = np.sin(ang)
    cos64 = np.concatenate(                                      # [B, 64, S]
        [cos32[:, 0:16], cos32[:, 0:16], cos32[:, 16:32], cos32[:, 16:32]],
        axis=1)
    sin64 = np.concatenate(
        [-sin32[:, 0:16], sin32[:, 0:16],
         -sin32[:, 16:32], sin32[:, 16:32]], axis=1)
    costab = np.tile(cos64, (1, 2, 1)).astype(np.float32)        # [B, 128, S]
    sintab = np.tile(sin64, (1, 2, 1)).astype(np.float32)

    tri = (np.arange(128)[None, :] >= np.arange(128)[:, None]).astype(bf16)

    rr = np.arange(QSLICE)
    in_maps = []
    for c in range(N_CORES):
        b, g = c // GROUPS, c % GROUPS
        wsel = 512 * (rr // 128) + 4 * (rr % 128) + g
        in_maps.append({
            "xT": np.asarray(xT[b]),
            "wqkvT": wslices[g],
            "woT": woT,
            "costab": costab[b].astype(bf16),
            "sintab": sintab[b].astype(bf16),
            "tri": tri,
            "wsel": wsel.astype(np.int32).reshape(QSLICE, 1),
        })
    return in_maps


def _assemble(results):
    out = np.empty((B, S, D), dtype=np.float32)
    for b in range(B):
        fullT = np.concatenate(
            [results[b * GROUPS + g]["finT"] for g in range(GROUPS)], axis=1)
        out[b] = fullT.T
    return out


_NC_CACHE = {}


def run(inputs, trace=False, **kw):
    if "nc" not in _NC_CACHE:
        _NC_CACHE["nc"] = _build()
    nc = _NC_CACHE["nc"]
    in_maps = _host_prep(**inputs)
    res = run_bass_kernel_spmd(
        nc, in_maps, core_ids=list(range(N_CORES)), trace=trace, **kw)
    return _assemble(res.results), res


def kernel(**inputs):
    out, _ = run(inputs, trace=False)
    return out


# revision 16
# speedup vs baseline: 1.1401x; 1.1401x over previous
"""Multi-head self-attention (RoPE, causal) on 8 trn2 NeuronCores.

Sharding: core c -> batch b = c // 4, head group g = c % 4 (4 heads each).

Per-core design (v2, W-stationary projection):
  - QKV projection runs W-stationary: lhsT = W^T chunk [128 d, 128 cols],
    rhs = x^T [128 d, 512 tokens], so Q^T/K^T/V^T come out already
    transposed ([feature, token]).  W columns are ordered per head-pair
    [q01 k01 v01 q23 k23 v23] and Q/K features are host-permuted to
    rotate-half order (even dims first), so RoPE becomes full-tile
    multiplies with per-token cos/sin tables (host-computed from
    token_positions - no gather) plus +-32-partition swaps.
  - scores^T = K^T-chunk x Q (k on partitions) into [128,1024] PSUM
    tiles, exp on ACT -> per-j et tiles (SBUF, column-offset 128j),
    diagonal masked by a lower-tri multiply.
  - V'-stationary matmuls (V plus a ones-column so softmax sums ride as
    row 64) accumulate DIRECTLY IN PSUM per 512-token q-chunk; only two
    q-chunk banks are live at a time (qc0/qc1 open first; when one
    closes it is normalized and the next opens, replaying buffered et
    tiles).  Normalize = DVE reciprocal of the sums row + gpsimd
    partition-broadcast + one multiply -> attT.
  - Projection of heads 2,3 + V transposes + out-projection pass 1 are
    emitted as PE filler between score/V-matmul steps, since the
    attention phase is ACT(exp)-paced.
  - AllGather (4-core batch group, split per head-pair) of attT; a
    per-core index gather (wsel) extracts this core's 512-token
    q-slice; two-pass output projection with the first AG consumed
    early.
Host only reshapes/permutes/casts inputs and concatenates outputs.
"""

import sys

for _p in ("/opt/trn_rl_repo",):
    if _p not in sys.path:
        sys.path.append(_p)

import numpy as np
import ml_dtypes

import concourse.bass as bass
import concourse.mybir as mybir
import concourse.tile as tile
from concourse import bacc
from concourse.bass import ds, ts
from concourse.bass_utils import run_bass_kernel_spmd
from concourse.masks import make_identity

BF16 = mybir.dt.bfloat16
F32 = mybir.dt.float32
I32 = mybir.dt.int32

B, S, D = 2, 2048, 1024
H, DK = 16, 64
THETA = 10000.0
N_CORES = 8
GROUPS = 4          # head groups (cores) per batch
HPC = H // GROUPS   # heads per core = 4
NSC = S // 128      # 16 token chunks
NQC = S // 512      # 4 q column-chunks
QSLICE = S // GROUPS
MUL = mybir.AluOpType.mult
ADD = mybir.AluOpType.add
EXP = mybir.ActivationFunctionType.Exp


def _build():
    nc = bacc.Bacc("TRN2", num_devices=N_CORES)

    xT = nc.dram_tensor("xT", [D, S], BF16, kind="ExternalInput")
    wqkvT = nc.dram_tensor("wqkvT", [D, 6 * 128], BF16, kind="ExternalInput")
    woT = nc.dram_tensor("woT", [D, D], BF16, kind="ExternalInput")
    costab = nc.dram_tensor("costab", [128, S], BF16, kind="ExternalInput")
    sintab = nc.dram_tensor("sintab", [128, S], BF16, kind="ExternalInput")
    tri = nc.dram_tensor("tri", [128, 128], BF16, kind="ExternalInput")
    wsel = nc.dram_tensor("wsel", [QSLICE, 1], I32, kind="ExternalInput")
    finT = nc.dram_tensor("finT", [D, QSLICE], F32, kind="ExternalOutput")

    with tile.TileContext(nc) as tc:
        with (
            tc.tile_pool(name="const", bufs=1) as constp,
            tc.tile_pool(name="wts", bufs=1) as wtsp,
            tc.tile_pool(name="seq", bufs=1) as seqp,
            tc.tile_pool(name="ropet", bufs=2) as ropet,
            tc.tile_pool(name="ett", bufs=1) as ett,
            tc.tile_pool(name="attp", bufs=1) as attp,
            tc.tile_pool(name="nrm", bufs=2) as nrmp,
            tc.tile_pool(name="xtp", bufs=3) as xtp,
            tc.tile_pool(name="spp", bufs=3, space="PSUM") as spp,
            tc.tile_pool(name="partp", bufs=2, space="PSUM") as partp,
            tc.tile_pool(name="dram", bufs=1, space="DRAM") as dramp,
        ):
            # ---------------- constants + resident weights ----------------
            ident = constp.tile([128, 128], BF16)
            make_identity(nc, ident[:])
            tri_t = constp.tile([128, 128], BF16)
            nc.sync.dma_start(out=tri_t[:], in_=tri[:])

            wt = wtsp.tile([128, 8, 6, 128], BF16)   # W^T [dchunk, wchunk]
            for wc in (0, 1):
                nc.sync.dma_start(
                    out=wt[:, :, wc, :],
                    in_=wqkvT[:, ts(wc, 128)].rearrange(
                        "(k p) c -> p k c", p=128),
                )
            xa = wtsp.tile([128, 8, S], BF16)     # resident x^T
            for k in range(8):
                nc.sync.dma_start(
                    out=xa[:, k, 0:1024], in_=xT[ts(k, 128), 0:1024])
            cost = wtsp.tile([128, S], BF16)
            nc.sync.dma_start(out=cost[:], in_=costab[:])
            sint = wtsp.tile([128, S], BF16)
            nc.sync.dma_start(out=sint[:], in_=sintab[:])
            for k in range(8):
                nc.sync.dma_start(
                    out=xa[:, k, 1024:2048], in_=xT[ts(k, 128), 1024:2048])
            for wc in (2, 3, 4, 5):
                nc.sync.dma_start(
                    out=wt[:, :, wc, :],
                    in_=wqkvT[:, ts(wc, 128)].rearrange(
                        "(k p) c -> p k c", p=128),
                )
            wo = wtsp.tile([128, 8, D], BF16)
            for k in range(8):
                nc.sync.dma_start(out=wo[:, k, :], in_=woT[ts(k, 128), :])

            # persistent per-core tensors
            qt = seqp.tile([128, 2, S], BF16)   # Q^T  [pair(2x64 feat), q]
            kt = seqp.tile([128, 2, S], BF16)   # K^T
            vv = seqp.tile([128, NSC, HPC, DK + 1], BF16)  # V + ones col
            nc.vector.memset(vv[:, :, :, DK:DK + 1], 1.0)
            attT = attp.tile([128, 2, S], BF16)
            rwo = attp.tile([128, 8, QSLICE], BF16)
            fe = attp.tile([128, 8, QSLICE], F32)
            wsel_sb = attp.tile([128, 4], I32)

            # per-j exp tiles: column q stored at (q - 128j)
            et = [ett.tile([128, S - 128 * j], BF16, name=f"et{j}")
                  for j in range(NSC)]

            # ------------- building blocks -------------
            def emit_proj(wc, tb):
                """Project wchunk wc for token block tb -> PSUM tile."""
                pj = spp.tile([128, 512], F32, space="PSUM", tag="big",
                              name="pj")
                for k in range(8):
                    nc.tensor.matmul(
                        pj[:], lhsT=wt[:, k, wc, :],
                        rhs=xa[:, k, ts(tb, 512)],
                        start=(k == 0), stop=(k == 7),
                    )
                return pj

            shuf16 = [i ^ 16 for i in range(32)]

            def emit_rope(wc, tb, pj):
                """RoPE pj -> qt/kt pair (wc in {0,1,3,4}).

                Features are quadrant-packed [16 even | 16 odd] so the
                rotate partner sits 16 partitions away within each
                32-partition quadrant; stream_shuffle does the swap."""
                dst = qt if wc in (0, 3) else kt
                pr = 0 if wc in (0, 1) else 1
                xsw = ropet.tile([128, 512], F32, tag="xsw")
                nc.vector.stream_shuffle(xsw[:], pj[:], shuf16)
                a = ropet.tile([128, 512], BF16, tag="ra")
                b = ropet.tile([128, 512], BF16, tag="rb")
                nc.vector.tensor_tensor(a[:], cost[:, ts(tb, 512)], pj[:],
                                        MUL)
                nc.vector.tensor_tensor(b[:], sint[:, ts(tb, 512)], xsw[:],
                                        MUL)
                nc.vector.tensor_tensor(
                    dst[:, pr, ts(tb, 512)], a[:], b[:], ADD)

            def emit_vchunk(wc, tb):
                """Project V wchunk + transpose into vv chunks 4tb..4tb+3."""
                h0 = 0 if wc == 2 else 2
                pj = emit_proj(wc, tb)
                vT = ropet.tile([128, 512], BF16, tag="vT")
                nc.vector.tensor_copy(vT[:], pj[:])
                for t in range(4):
                    tp = spp.tile([128, 128], BF16, space="PSUM", tag="big",
                                  name="tp")
                    nc.tensor.transpose(tp[:], vT[:, ts(t, 128)], ident[:])
                    nc.vector.tensor_copy(
                        vv[:, tb * 4 + t, h0:h0 + 2, 0:DK],
                        tp[:].rearrange("p (h e) -> p h e", h=2),
                    )

            # ---- filler queue (PE work consumed inside attention) ----
            fillers = []

            def drain(n=1):
                for _ in range(min(n, len(fillers))):
                    fillers.pop(0)()

            # early rendezvous: tiny AllGather absorbs cross-core launch
            # skew while the input DMAs stream, so the real AllGathers
            # later don't pay it
            syin = dramp.tile([16, 128], BF16, name="syin")
            syout = dramp.tile([64, 128], BF16, name="syout")
            nc.sync.dma_start(out=syin[:], in_=tri_t[0:16, :])
            nc.gpsimd.collective_compute(
                "AllGather", mybir.AluOpType.bypass,
                ins=[syin[:]], outs=[syout[:]],
                replica_groups=[[0, 1, 2, 3], [4, 5, 6, 7]],
            )

            # ------------- attention -------------
            agin = [dramp.tile([4 * 128, QSLICE], BF16, name=f"agin{p}")
                    for p in range(2)]
            agout = [dramp.tile([4 * 512, QSLICE], BF16, name=f"agout{p}")
                     for p in range(2)]

            def emit_scores(h, j):
                pr, hf = h // 2, (h % 2) * DK
                for qh in range(2):
                    q0 = max(128 * j, 1024 * qh)
                    q1 = 1024 * (qh + 1)
                    if q0 >= q1:
                        continue
                    sp = spp.tile([128, 1024], F32, space="PSUM", tag="big")
                    for qq in (1024 * qh, 1024 * qh + 512):
                        a, bnd = max(q0, qq), min(q1, qq + 512)
                        if a >= bnd:
                            continue
                        nc.tensor.matmul(
                            sp[:, ds(a - 1024 * qh, bnd - a)],
                            lhsT=kt[ds(hf, DK), pr, ts(j, 128)],
                            rhs=qt[ds(hf, DK), pr, ds(a, bnd - a)],
                            start=True, stop=True,
                        )
                    nc.scalar.activation(
                        et[j][:, ds(q0 - 128 * j, q1 - q0)],
                        sp[:, ds(q0 - 1024 * qh, q1 - q0)], EXP)
                # mask the diagonal block (q < k -> 0)
                nc.vector.tensor_tensor(
                    et[j][:, 0:128], et[j][:, 0:128], tri_t[:], MUL)

            def vmm(h, qc, j, start):
                a = max(512 * qc, 128 * j)
                bnd = 512 * (qc + 1)
                nc.tensor.matmul(
                    parts[qc][:, ds(a - 512 * qc, bnd - a)],
                    lhsT=vv[:, j, h, :],
                    rhs=et[j][:, ds(a - 128 * j, bnd - a)],
                    start=start, stop=(j == 4 * qc + 3),
                    skip_group_check=True,
                )

            def emit_normalize(h, qc):
                pr, hf = h // 2, (h % 2) * DK
                part = parts[qc]
                srow = nrmp.tile([1, 512], F32, tag="srow")
                nc.vector.tensor_copy(srow[:], part[ds(DK, 1), :])
                nc.vector.reciprocal_approx_fast(srow[:], srow[:])
                sumb = nrmp.tile([64, 512], F32, tag="sumb")
                nc.gpsimd.partition_broadcast(sumb[:], srow[:])
                nc.vector.tensor_tensor(
                    attT[ds(hf, DK), pr, ts(qc, 512)],
                    part[0:DK, :], sumb[:], MUL)
                parts[qc] = None
                if h % 2 == 1:   # pair piece complete -> stage for AG
                    p = h // 2
                    nc.sync.dma_start(
                        out=agin[p][:, :].rearrange(
                            "(d j) q -> d j q", j=GROUPS)[:, qc, :],
                        in_=attT[:, p, ts(qc, 512)],
                    )
                    if qc == NQC - 1:
                        nc.gpsimd.collective_compute(
                            "AllGather",
                            mybir.AluOpType.bypass,
                            ins=[agin[p][:]],
                            outs=[agout[p][:]],
                            replica_groups=[[0, 1, 2, 3], [4, 5, 6, 7]],
                        )

            def open_group(h, qc, jmax):
                """(Re)open accumulation group qc, replaying et j=0..jmax."""
                parts[qc] = partp.tile([DK + 1, 512], F32, space="PSUM",
                                       tag="part", name=f"part{qc}")
                for j in range(0, min(jmax, 4 * qc + 3) + 1):
                    vmm(h, qc, j, start=(j == 0))

            def emit_vstep(h, j):
                """V-matmuls for step j + group closures."""
                for qc in (0, 1, 2, 3):
                    if parts[qc] is not None and j <= 4 * qc + 3:
                        vmm(h, qc, j, start=(j == 0))
                for qc in (0, 1, 2, 3):
                    if parts[qc] is not None and j == 4 * qc + 3:
                        emit_normalize(h, qc)
                        if qc + 2 <= 3:
                            open_group(h, qc + 2, j)

            # ------------- out-projection pieces -------------
            for c in range(4):
                nc.sync.dma_start(
                    out=wsel_sb[:, c:c + 1], in_=wsel[ts(c, 128), :])

            def emit_gather(dp):
                nc.gpsimd.indirect_dma_start(
                    out=rwo[:, dp, :],
                    out_offset=None,
                    in_=agout[dp % 2][:],
                    in_offset=bass.IndirectOffsetOnAxis(
                        ap=wsel_sb[:, dp // 2:dp // 2 + 1], axis=0),
                )

            def emit_pass1(ec):
                fp = spp.tile([128, 512], F32, space="PSUM", tag="big")
                for i, dp in enumerate((0, 2, 4, 6)):
                    nc.tensor.matmul(
                        fp[:], lhsT=wo[:, dp, ts(ec, 128)], rhs=rwo[:, dp, :],
                        start=(i == 0), stop=(i == 3),
                    )
                nc.vector.tensor_copy(fe[:, ec, :], fp[:])

            # ================= emission schedule =================
            # Phase A: Q,K of heads 0,1 (wc 0,1) through the big PSUM pool
            for wc in (0, 1):
                for tb in range(4):
                    emit_rope(wc, tb, emit_proj(wc, tb))

            # fillers: q23, k23 rope quanta then v23 chunks
            for wc in (3, 4):
                for tb in range(4):
                    fillers.append(
                        lambda wc=wc, tb=tb: emit_rope(wc, tb,
                                                       emit_proj(wc, tb)))

            for h in range(HPC):
                parts = [None, None, None, None]
                open_group(h, 0, -1)
                open_group(h, 1, -1)
                for j in range(NSC):
                    if h == 0 and j % 4 == 0:
                        emit_vchunk(2, j // 4)
                    if h == 1 and j % 4 == 0:
                        emit_vchunk(5, j // 4)
                    emit_scores(h, j)
                    if j > 0:
                        emit_vstep(h, j - 1)
                    if h == 0 and j >= 1:
                        drain(1)
                    if h == 3 and j >= 8:
                        drain(1)
                emit_vstep(h, NSC - 1)
                for qc in (0, 1, 2, 3):   # close any remaining groups
                    if parts[qc] is not None:
                        emit_normalize(h, qc)
                if h == 1:
                    # AG#0 just queued; gathers for even d-chunks + pass1
                    for dp in (0, 2, 4, 6):
                        emit_gather(dp)
                    for ec in range(8):
                        fillers.append(lambda ec=ec: emit_pass1(ec))

            drain(len(fillers))

            # ------------- tail: AG#1 + pass 2 -------------
            for dp in (1, 3, 5, 7):
                emit_gather(dp)
            for ec in range(8):
                fp = spp.tile([128, 512], F32, space="PSUM", tag="big")
                for i, dp in enumerate((1, 3, 5, 7)):
                    nc.tensor.matmul(
                        fp[:], lhsT=wo[:, dp, ts(ec, 128)], rhs=rwo[:, dp, :],
                        start=(i == 0), stop=(i == 3),
                    )
                fin_sb = xtp.tile([128, QSLICE], F32, tag="fin")
                nc.vector.tensor_tensor(fin_sb[:], fe[:, ec, :], fp[:], ADD)
                nc.sync.dma_start(out=finT[ts(ec, 128), :], in_=fin_sb[:])

    nc.compile()
    return nc


def _host_prep(x, token_positions, W_qkv, W_o):
    bf16 = ml_dtypes.bfloat16
    xT = np.ascontiguousarray(np.transpose(x, (0, 2, 1))).astype(bf16)

    # quadrant-packed rotate permutation: per 32-row quadrant the even
    # dims of 16 pairs then their odd partners (stream_shuffle swaps at
    # +-16 partitions within a quadrant)
    perm64 = np.concatenate([
        np.arange(0, 32, 2), np.arange(1, 32, 2),
        np.arange(32, 64, 2), np.arange(33, 64, 2),
    ])

    wq = W_qkv[0 * D:1 * D] * np.float32(1.0 / np.sqrt(DK))
    wk = W_qkv[1 * D:2 * D]
    wv = W_qkv[2 * D:3 * D]
    wslices = []
    for g in range(GROUPS):
        base = g * HPC * DK
        cols = []
        for pr in range(2):           # head pairs
            hA, hB = base + (2 * pr) * DK, base + (2 * pr + 1) * DK
            qrows = np.concatenate(
                [wq[hA + perm64], wq[hB + perm64]], axis=0)      # [128, D]
            krows = np.concatenate(
                [wk[hA + perm64], wk[hB + perm64]], axis=0)
            vrows = np.concatenate(
                [wv[hA:hA + DK], wv[hB:hB + DK]], axis=0)
            cols += [qrows, krows, vrows]
        wsl = np.concatenate(cols, axis=0)                       # [768, D]
        wslices.append(np.ascontiguousarray(wsl.T).astype(bf16))  # [D, 768]

    woT = np.ascontiguousarray(W_o.T).astype(bf16)

    # per-token cos/sin tables in quadrant-packed transposed layout
    idx = np.arange(DK // 2, dtype=np.float64)
    freqs = 1.0 / (THETA ** (2.0 * idx / DK))                    # [32]
    pos = np.asarray(token_positions).astype(np.float64)         # [B, S]
    ang = pos[:, None, :] * freqs[None, :, None]                 # [B, 32, S]
    cos32 = np.cos(ang)
    sin32 = np.sin(ang)
    cos64 = np.concatenate(                                      # [B, 64, S]
        [cos32[:, 0:16], cos32[:, 0:16], cos32[:, 16:32], cos32[:, 16:32]],
        axis=1)
    sin64 = np.concatenate(
        [-sin32[:, 0:16], sin32[:, 0:16],
         -sin32[:, 16:32], sin32[:, 16:32]], axis=1)
    costab = np.tile(cos64, (1, 2, 1)).astype(np.float32)        # [B, 128, S]
    sintab = np.tile(sin64, (1, 2, 1)).astype(np.float32)

    tri = (np.arange(128)[None, :] >= np.arange(128)[:, None]).astype(bf16)

    rr = np.arange(QSLICE)
    in_maps = []
    for c in range(N_CORES):
        b, g = c // GROUPS, c % GROUPS
        wsel = 512 * (rr // 128) + 4 * (rr % 128) + g
        in_maps.append({
            "xT": np.asarray(xT[b]),
            "wqkvT": wslices[g],
            "woT": woT,
            "costab": costab[b].astype(bf16),
            "sintab": sintab[b].astype(bf16),
            "tri": tri,
            "wsel": wsel.astype(np.int32).reshape(QSLICE, 1),
        })
    return in_maps


def _assemble(results):
    out = np.empty((B, S, D), dtype=np.float32)
    for b in range(B):
        fullT = np.concatenate(
            [results[b * GROUPS + g]["finT"] for g in range(GROUPS)], axis=1)
        out[b] = fullT.T
    return out


_NC_CACHE = {}


def run(inputs, trace=False, **kw):
    if "nc" not in _NC_CACHE:
        _NC_CACHE["nc"] = _build()
    nc = _NC_CACHE["nc"]
    in_maps = _host_prep(**inputs)
    res = run_bass_kernel_spmd(
        nc, in_maps, core_ids=list(range(N_CORES)), trace=trace, **kw)
    return _assemble(res.results), res


def kernel(**inputs):
    out, _ = run(inputs, trace=False)
    return out


# revision 18
# speedup vs baseline: 1.1672x; 1.0237x over previous
"""Multi-head self-attention (RoPE, causal) on 8 trn2 NeuronCores.

Sharding: core c -> batch b = c // 4, head group g = c % 4 (4 heads each).

Per-core design (v2, W-stationary projection):
  - QKV projection runs W-stationary: lhsT = W^T chunk [128 d, 128 cols],
    rhs = x^T [128 d, 512 tokens], so Q^T/K^T/V^T come out already
    transposed ([feature, token]).  W columns are ordered per head-pair
    [q01 k01 v01 q23 k23 v23] and Q/K features are host-permuted to
    rotate-half order (even dims first), so RoPE becomes full-tile
    multiplies with per-token cos/sin tables (host-computed from
    token_positions - no gather) plus +-32-partition swaps.
  - scores^T = K^T-chunk x Q (k on partitions) into [128,1024] PSUM
    tiles, exp on ACT -> per-j et tiles (SBUF, column-offset 128j),
    diagonal masked by a lower-tri multiply.
  - V'-stationary matmuls (V plus a ones-column so softmax sums ride as
    row 64) accumulate DIRECTLY IN PSUM per 512-token q-chunk; only two
    q-chunk banks are live at a time (qc0/qc1 open first; when one
    closes it is normalized and the next opens, replaying buffered et
    tiles).  Normalize = DVE reciprocal of the sums row + gpsimd
    partition-broadcast + one multiply -> attT.
  - Projection of heads 2,3 + V transposes + out-projection pass 1 are
    emitted as PE filler between score/V-matmul steps, since the
    attention phase is ACT(exp)-paced.
  - AllGather (4-core batch group, split per head-pair) of attT; a
    per-core index gather (wsel) extracts this core's 512-token
    q-slice; two-pass output projection with the first AG consumed
    early.
Host only reshapes/permutes/casts inputs and concatenates outputs.
"""

import sys

for _p in ("/opt/trn_rl_repo",):
    if _p not in sys.path:
        sys.path.append(_p)

import numpy as np
import ml_dtypes

import concourse.bass as bass
import concourse.mybir as mybir
import concourse.tile as tile
from concourse import bacc
from concourse.bass import ds, ts
from concourse.bass_utils import run_bass_kernel_spmd
from concourse.masks import make_identity

BF16 = mybir.dt.bfloat16
F32 = mybir.dt.float32
I32 = mybir.dt.int32

B, S, D = 2, 2048, 1024
H, DK = 16, 64
THETA = 10000.0
N_CORES = 8
GROUPS = 4          # head groups (cores) per batch
HPC = H // GROUPS   # heads per core = 4
NSC = S // 128      # 16 token chunks
NQC = S // 512      # 4 q column-chunks
QSLICE = S // GROUPS
MUL = mybir.AluOpType.mult
ADD = mybir.AluOpType.add
EXP = mybir.ActivationFunctionType.Exp


def _build():
    nc = bacc.Bacc("TRN2", num_devices=N_CORES)

    xT = nc.dram_tensor("xT", [D, S], BF16, kind="ExternalInput")
    wqkvT = nc.dram_tensor("wqkvT", [D, 6 * 128], BF16, kind="ExternalInput")
    woT = nc.dram_tensor("woT", [D, D], BF16, kind="ExternalInput")
    costab = nc.dram_tensor("costab", [128, S], BF16, kind="ExternalInput")
    sintab = nc.dram_tensor("sintab", [128, S], BF16, kind="ExternalInput")
    tri = nc.dram_tensor("tri", [128, 128], BF16, kind="ExternalInput")
    wsel = nc.dram_tensor("wsel", [QSLICE, 1], I32, kind="ExternalInput")
    finT = nc.dram_tensor("finT", [D, QSLICE], F32, kind="ExternalOutput")

    with tile.TileContext(nc) as tc:
        with (
            tc.tile_pool(name="const", bufs=1) as constp,
            tc.tile_pool(name="wts", bufs=1) as wtsp,
            tc.tile_pool(name="seq", bufs=1) as seqp,
            tc.tile_pool(name="ropet", bufs=2) as ropet,
            tc.tile_pool(name="ett", bufs=1) as ett,
            tc.tile_pool(name="attp", bufs=1) as attp,
            tc.tile_pool(name="nrm", bufs=2) as nrmp,
            tc.tile_pool(name="xtp", bufs=3) as xtp,
            tc.tile_pool(name="spp", bufs=3, space="PSUM") as spp,
            tc.tile_pool(name="partp", bufs=2, space="PSUM") as partp,
            tc.tile_pool(name="dram", bufs=1, space="DRAM") as dramp,
        ):
            # ---------------- constants + resident weights ----------------
            ident = constp.tile([128, 128], BF16)
            make_identity(nc, ident[:])
            tri_t = constp.tile([128, 128], BF16)
            nc.sync.dma_start(out=tri_t[:], in_=tri[:])

            wt = wtsp.tile([128, 8, 6, 128], BF16)   # W^T [dchunk, wchunk]
            for wc in (0, 1):
                nc.sync.dma_start(
                    out=wt[:, :, wc, :],
                    in_=wqkvT[:, ts(wc, 128)].rearrange(
                        "(k p) c -> p k c", p=128),
                )
            xa = wtsp.tile([128, 8, S], BF16)     # resident x^T
            for k in range(8):
                nc.sync.dma_start(
                    out=xa[:, k, 0:1024], in_=xT[ts(k, 128), 0:1024])
            cost = wtsp.tile([128, S], BF16)
            nc.sync.dma_start(out=cost[:], in_=costab[:])
            sint = wtsp.tile([128, S], BF16)
            nc.sync.dma_start(out=sint[:], in_=sintab[:])
            for k in range(8):
                nc.sync.dma_start(
                    out=xa[:, k, 1024:2048], in_=xT[ts(k, 128), 1024:2048])
            for wc in (2, 3, 4, 5):
                nc.sync.dma_start(
                    out=wt[:, :, wc, :],
                    in_=wqkvT[:, ts(wc, 128)].rearrange(
                        "(k p) c -> p k c", p=128),
                )
            wo = wtsp.tile([128, 8, D], BF16)
            for k in range(8):
                nc.sync.dma_start(out=wo[:, k, :], in_=woT[ts(k, 128), :])

            # persistent per-core tensors
            qt = seqp.tile([128, 2, S], BF16)   # Q^T  [pair(2x64 feat), q]
            kt = seqp.tile([128, 2, S], BF16)   # K^T
            vv = seqp.tile([128, NSC, HPC, DK + 1], BF16)  # V + ones col
            nc.vector.memset(vv[:, :, :, DK:DK + 1], 1.0)
            attT = attp.tile([128, 2, S], BF16)
            rwo = attp.tile([128, 8, QSLICE], BF16)
            fe = attp.tile([128, 8, QSLICE], F32)
            wsel_sb = attp.tile([128, 4], I32)

            # per-j exp tiles: column q stored at (q - 128j)
            et = [ett.tile([128, S - 128 * j], BF16, name=f"et{j}")
                  for j in range(NSC)]

            # ------------- building blocks -------------
            def emit_proj(wc, tb):
                """Project wchunk wc for token block tb -> PSUM tile."""
                pj = spp.tile([128, 512], F32, space="PSUM", tag="big",
                              name="pj")
                for k in range(8):
                    nc.tensor.matmul(
                        pj[:], lhsT=wt[:, k, wc, :],
                        rhs=xa[:, k, ts(tb, 512)],
                        start=(k == 0), stop=(k == 7),
                    )
                return pj

            shuf16 = [i ^ 16 for i in range(32)]

            def emit_rope(wc, tb, pj):
                """RoPE pj -> qt/kt pair (wc in {0,1,3,4}).

                Features are quadrant-packed [16 even | 16 odd] so the
                rotate partner sits 16 partitions away within each
                32-partition quadrant; stream_shuffle does the swap."""
                dst = qt if wc in (0, 3) else kt
                pr = 0 if wc in (0, 1) else 1
                xsw = ropet.tile([128, 512], F32, tag="xsw")
                nc.vector.stream_shuffle(xsw[:], pj[:], shuf16)
                a = ropet.tile([128, 512], BF16, tag="ra")
                b = ropet.tile([128, 512], BF16, tag="rb")
                nc.vector.tensor_tensor(a[:], cost[:, ts(tb, 512)], pj[:],
                                        MUL)
                nc.vector.tensor_tensor(b[:], sint[:, ts(tb, 512)], xsw[:],
                                        MUL)
                nc.vector.tensor_tensor(
                    dst[:, pr, ts(tb, 512)], a[:], b[:], ADD)

            def emit_vproj(wc, tb):
                """Project V wchunk tb -> vT staging tile (copy on DVE)."""
                pj = emit_proj(wc, tb)
                vT = ropet.tile([128, 512], BF16, tag=f"vT{tb % 2}")
                nc.vector.tensor_copy(vT[:], pj[:])
                return vT

            def emit_vtrans(wc, tb, vT):
                """Transpose staged V block into vv chunks 4tb..4tb+3."""
                h0 = 0 if wc == 2 else 2
                for t in range(4):
                    tp = spp.tile([128, 128], BF16, space="PSUM", tag="big",
                                  name="tp")
                    nc.tensor.transpose(tp[:], vT[:, ts(t, 128)], ident[:])
                    nc.vector.tensor_copy(
                        vv[:, tb * 4 + t, h0:h0 + 2, 0:DK],
                        tp[:].rearrange("p (h e) -> p h e", h=2),
                    )

            # early rendezvous: tiny AllGather absorbs cross-core launch
            # skew while the input DMAs stream, so the real AllGathers
            # later don't pay it
            syin = dramp.tile([16, 128], BF16, name="syin")
            syout = dramp.tile([64, 128], BF16, name="syout")
            nc.sync.dma_start(out=syin[:], in_=tri_t[0:16, :])
            nc.gpsimd.collective_compute(
                "AllGather", mybir.AluOpType.bypass,
                ins=[syin[:]], outs=[syout[:]],
                replica_groups=[[0, 1, 2, 3], [4, 5, 6, 7]],
            )

            # ------------- attention -------------
            agin = [dramp.tile([4 * 128, QSLICE], BF16, name=f"agin{p}")
                    for p in range(2)]
            agout = [dramp.tile([4 * 512, QSLICE], BF16, name=f"agout{p}")
                     for p in range(2)]

            def emit_scores(h, j):
                pr, hf = h // 2, (h % 2) * DK
                for qh in range(2):
                    q0 = max(128 * j, 1024 * qh)
                    q1 = 1024 * (qh + 1)
                    if q0 >= q1:
                        continue
                    sp = spp.tile([128, 1024], F32, space="PSUM", tag="big")
                    for qq in (1024 * qh, 1024 * qh + 512):
                        a, bnd = max(q0, qq), min(q1, qq + 512)
                        if a >= bnd:
                            continue
                        nc.tensor.matmul(
                            sp[:, ds(a - 1024 * qh, bnd - a)],
                            lhsT=kt[ds(hf, DK), pr, ts(j, 128)],
                            rhs=qt[ds(hf, DK), pr, ds(a, bnd - a)],
                            start=True, stop=True,
                        )
                    nc.scalar.activation(
                        et[j][:, ds(q0 - 128 * j, q1 - q0)],
                        sp[:, ds(q0 - 1024 * qh, q1 - q0)], EXP)
                # mask the diagonal block (q < k -> 0)
                nc.vector.tensor_tensor(
                    et[j][:, 0:128], et[j][:, 0:128], tri_t[:], MUL)

            def vmm(h, qc, j, start):
                a = max(512 * qc, 128 * j)
                bnd = 512 * (qc + 1)
                nc.tensor.matmul(
                    parts[qc][:, ds(a - 512 * qc, bnd - a)],
                    lhsT=vv[:, j, h, :],
                    rhs=et[j][:, ds(a - 128 * j, bnd - a)],
                    start=start, stop=(j == 4 * qc + 3),
                    skip_group_check=True,
                )

            def emit_normalize(h, qc):
                pr, hf = h // 2, (h % 2) * DK
                part = parts[qc]
                srow = nrmp.tile([1, 512], F32, tag="srow")
                nc.vector.tensor_copy(srow[:], part[ds(DK, 1), :])
                nc.vector.reciprocal_approx_fast(srow[:], srow[:])
                sumb = nrmp.tile([64, 512], F32, tag="sumb")
                nc.gpsimd.partition_broadcast(sumb[:], srow[:])
                nc.vector.tensor_tensor(
                    attT[ds(hf, DK), pr, ts(qc, 512)],
                    part[0:DK, :], sumb[:], MUL)
                parts[qc] = None
                if h % 2 == 1:   # pair piece complete -> stage for AG
                    p = h // 2
                    nc.sync.dma_start(
                        out=agin[p][:, :].rearrange(
                            "(d j) q -> d j q", j=GROUPS)[:, qc, :],
                        in_=attT[:, p, ts(qc, 512)],
                    )
                    if qc == NQC - 1:
                        nc.gpsimd.collective_compute(
                            "AllGather",
                            mybir.AluOpType.bypass,
                            ins=[agin[p][:]],
                            outs=[agout[p][:]],
                            replica_groups=[[0, 1, 2, 3], [4, 5, 6, 7]],
                        )

            def open_group(h, qc, jmax):
                """(Re)open accumulation group qc, replaying et j=0..jmax."""
                parts[qc] = partp.tile([DK + 1, 512], F32, space="PSUM",
                                       tag="part", name=f"part{qc}")
                for j in range(0, min(jmax, 4 * qc + 3) + 1):
                    vmm(h, qc, j, start=(j == 0))

            def emit_vstep(h, j):
                """V-matmuls for step j + group closures."""
                for qc in (0, 1, 2, 3):
                    if parts[qc] is not None and j <= 4 * qc + 3:
                        vmm(h, qc, j, start=(j == 0))
                for qc in (0, 1, 2, 3):
                    if parts[qc] is not None and j == 4 * qc + 3:
                        emit_normalize(h, qc)
                        if qc + 2 <= 3:
                            open_group(h, qc + 2, j)

            # ------------- out-projection pieces -------------
            for c in range(4):
                nc.sync.dma_start(
                    out=wsel_sb[:, c:c + 1], in_=wsel[ts(c, 128), :])

            def emit_gather(dp):
                nc.gpsimd.indirect_dma_start(
                    out=rwo[:, dp, :],
                    out_offset=None,
                    in_=agout[dp % 2][:],
                    in_offset=bass.IndirectOffsetOnAxis(
                        ap=wsel_sb[:, dp // 2:dp // 2 + 1], axis=0),
                )

            def emit_pass1(ec):
                fp = spp.tile([128, 512], F32, space="PSUM", tag="big")
                for i, dp in enumerate((0, 2, 4, 6)):
                    nc.tensor.matmul(
                        fp[:], lhsT=wo[:, dp, ts(ec, 128)], rhs=rwo[:, dp, :],
                        start=(i == 0), stop=(i == 3),
                    )
                nc.vector.tensor_copy(fe[:, ec, :], fp[:])

            # ================= emission schedule =================
            # Phase A: Q,K of heads 0,1 (wc 0,1) through the big PSUM pool
            for wc in (0, 1):
                for tb in range(4):
                    emit_rope(wc, tb, emit_proj(wc, tb))

            LAG = 2
            # filler schedule: {(h, j): [callables]} with v-chunk two-step
            # quanta pinned so vv chunks land just before their vmms
            sched = {}

            def at(h, j, fn):
                sched.setdefault((h, j), []).append(fn)

            vstage = {}
            for tb in range(4):
                def vp(wc=2, tb=tb):
                    vstage[(2, tb)] = emit_vproj(wc, tb)
                def vt(wc=2, tb=tb):
                    emit_vtrans(wc, tb, vstage[(2, tb)])
                at(0, max(4 * tb - 1, 0), vp)
                at(0, 4 * tb + 1, vt)
            for i, tb in enumerate(range(4)):     # q23 rope quanta
                at(0, 4 * i + 2, lambda tb=tb: emit_rope(
                    3, tb, emit_proj(3, tb)))
            for i, tb in enumerate(range(4)):     # k23 rope quanta
                at(1, 2 * i, lambda tb=tb: emit_rope(
                    4, tb, emit_proj(4, tb)))
            for tb in range(4):
                def vp5(wc=5, tb=tb):
                    vstage[(5, tb)] = emit_vproj(wc, tb)
                def vt5(wc=5, tb=tb):
                    emit_vtrans(wc, tb, vstage[(5, tb)])
                at(1, 2 * tb + 8, vp5)
                at(1, 2 * tb + 9, vt5)
            for ec in range(8):                   # pass1 during h3
                at(3, 2 * ec, lambda ec=ec: emit_pass1(ec))

            for h in range(HPC):
                parts = [None, None, None, None]
                open_group(h, 0, -1)
                open_group(h, 1, -1)
                for j in range(NSC + LAG):
                    if j < NSC:
                        emit_scores(h, j)
                    for fn in sched.pop((h, j), ()):
                        fn()
                    if j >= LAG:
                        emit_vstep(h, j - LAG)
                for qc in (0, 1, 2, 3):   # close any remaining groups
                    if parts[qc] is not None:
                        emit_normalize(h, qc)
                if h == 1:
                    # AG#0 just queued; gathers for even d-chunks
                    for dp in (0, 2, 4, 6):
                        emit_gather(dp)

            # ------------- tail: AG#1 + pass 2 -------------
            for dp in (1, 3, 5, 7):
                emit_gather(dp)
            for ec in range(8):
                fp = spp.tile([128, 512], F32, space="PSUM", tag="big")
                for i, dp in enumerate((1, 3, 5, 7)):
                    nc.tensor.matmul(
                        fp[:], lhsT=wo[:, dp, ts(ec, 128)], rhs=rwo[:, dp, :],
                        start=(i == 0), stop=(i == 3),
                    )
                fin_sb = xtp.tile([128, QSLICE], F32, tag="fin")
                nc.vector.tensor_tensor(fin_sb[:], fe[:, ec, :], fp[:], ADD)
                nc.sync.dma_start(out=finT[ts(ec, 128), :], in_=fin_sb[:])

    nc.compile()
    return nc


def _host_prep(x, token_positions, W_qkv, W_o):
    bf16 = ml_dtypes.bfloat16
    xT = np.ascontiguousarray(np.transpose(x, (0, 2, 1))).astype(bf16)

    # quadrant-packed rotate permutation: per 32-row quadrant the even
    # dims of 16 pairs then their odd partners (stream_shuffle swaps at
    # +-16 partitions within a quadrant)
    perm64 = np.concatenate([
        np.arange(0, 32, 2), np.arange(1, 32, 2),
        np.arange(32, 64, 2), np.arange(33, 64, 2),
    ])

    wq = W_qkv[0 * D:1 * D] * np.float32(1.0 / np.sqrt(DK))
    wk = W_qkv[1 * D:2 * D]
    wv = W_qkv[2 * D:3 * D]
    wslices = []
    for g in range(GROUPS):
        base = g * HPC * DK
        cols = []
        for pr in range(2):           # head pairs
            hA, hB = base + (2 * pr) * DK, base + (2 * pr + 1) * DK
            qrows = np.concatenate(
                [wq[hA + perm64], wq[hB + perm64]], axis=0)      # [128, D]
            krows = np.concatenate(
                [wk[hA + perm64], wk[hB + perm64]], axis=0)
            vrows = np.concatenate(
                [wv[hA:hA + DK], wv[hB:hB + DK]], axis=0)
            cols += [qrows, krows, vrows]
        wsl = np.concatenate(cols, axis=0)                       # [768, D]
        wslices.append(np.ascontiguousarray(wsl.T).astype(bf16))  # [D, 768]

    woT = np.ascontiguousarray(W_o.T).astype(bf16)

    # per-token cos/sin tables in quadrant-packed transposed layout
    idx = np.arange(DK // 2, dtype=np.float64)
    freqs = 1.0 / (THETA ** (2.0 * idx / DK))                    # [32]
    pos = np.asarray(token_positions).astype(np.float64)         # [B, S]
    ang = pos[:, None, :] * freqs[None, :, None]                 # [B, 32, S]
    cos32 = np.cos(ang)
    sin32 = np.sin(ang)
    cos64 = np.concatenate(                                      # [B, 64, S]
        [cos32[:, 0:16], cos32[:, 0:16], cos32[:, 16:32], cos32[:, 16:32]],
        axis=1)
    sin64 = np.concatenate(
        [-sin32[:, 0:16], sin32[:, 0:16],
         -sin32[:, 16:32], sin32[:, 16:32]], axis=1)
    costab = np.tile(cos64, (1, 2, 1)).astype(np.float32)        # [B, 128, S]
    sintab = np.tile(sin64, (1, 2, 1)).astype(np.float32)

    tri = (np.arange(128)[None, :] >= np.arange(128)[:, None]).astype(bf16)

    rr = np.arange(QSLICE)
    in_maps = []
    for c in range(N_CORES):
        b, g = c // GROUPS, c % GROUPS
        wsel = 512 * (rr // 128) + 4 * (rr % 128) + g
        in_maps.append({
            "xT": np.asarray(xT[b]),
            "wqkvT": wslices[g],
            "woT": woT,
            "costab": costab[b].astype(bf16),
            "sintab": sintab[b].astype(bf16),
            "tri": tri,
            "wsel": wsel.astype(np.int32).reshape(QSLICE, 1),
        })
    return in_maps


def _assemble(results):
    out = np.empty((B, S, D), dtype=np.float32)
    for b in range(B):
        fullT = np.concatenate(
            [results[b * GROUPS + g]["finT"] for g in range(GROUPS)], axis=1)
        out[b] = fullT.T
    return out


_NC_CACHE = {}


def run(inputs, trace=False, **kw):
    if "nc" not in _NC_CACHE:
        _NC_CACHE["nc"] = _build()
    nc = _NC_CACHE["nc"]
    in_maps = _host_prep(**inputs)
    res = run_bass_kernel_spmd(
        nc, in_maps, core_ids=list(range(N_CORES)), trace=trace, **kw)
    return _assemble(res.results), res


def kernel(**inputs):
    out, _ = run(inputs, trace=False)
    return out
